# revision 24
# baseline (speedup 1.0000x reference)
"""BandSplit layer Trainium2 kernel.

Computes, for input [16, 1000, 257]:
  - 28 frequency bands: 8 bands x 4 bins (bins 0..31), 12 x 8 (32..127),
    8 x 16 (128..255)  (bin 256 unused)
  - per-band layernorm over the band's bins (eps=1e-3), with per-band
    gamma/beta, then a per-band dense projection [c] -> [128] plus bias.
  - output [16, 1000, 28, 128]

Strategy: data-parallel over batch across 8 NeuronCores (2 batches =
2000 tokens per core).  gamma is folded into the dense weights and
beta/bias into a single per-output bias on the host, so the device does
plain layernorm + matmul.

DMA choreography (the kernel is HBM-write bound at ~400 GB/s/core):
  - x is repacked host-side into the exact SBUF image [128, 16, 257]
    so the prefetch runs in 3 DMAs with multi-KB descriptors (tiles 0-1
    first so compute starts early),
  - the scalar (Activation) HWDGE queue carries all loads, the sync
    queue carries only output stores (2 half-tile stores per 128-token
    tile) so the store stream is never stuck behind load issues,
  - per-band 1/c constants are built with memsets, not DMA.
Per 128-token tile: LN stats via free-dim reduces (DVE) -> normalize in
place (vector for the first tiles to shortcut the startup dependency
chain, GpSimd after) -> PE transpose to [bins, tok] -> 7 fp32r matmuls
(K=128 against the packed block-diagonal weights, N=512) in 2-bank
PSUM pairs -> drains alternating Scalar/Vector -> half-tile DMAs out.
"""

import sys

import numpy as np

for _p in ("/opt/trn_rl_repo", "/root/.axon_site/_ro/trn_rl_repo"):
    if _p not in sys.path:
        sys.path.append(_p)

EPS = 1e-3
D = 128
GROUPS = [(8, 4, 0), (12, 8, 32), (8, 16, 128)]  # (n_bands, bins_per_band, start_bin)
B, T, F = 16, 1000, 257
N_CORES = 8
TOK = B * T // N_CORES  # tokens per core = 2000
NB = sum(n for n, _, _ in GROUPS)  # 28 bands
OUT_COLS = NB * D  # 3584
P = 128
N_TILES = (TOK + P - 1) // P  # 16 (last tile holds 80 tokens)
N_CHUNK = 512  # matmul free-dim chunk (one PSUM bank)
N_CHUNKS = OUT_COLS // N_CHUNK  # 7
# Per-band layout: (ktile, krow0, c) per band; ktile 0 = bins 0..127,
# ktile 1 = bins 128..255. Output cols for band i are [i*128, (i+1)*128).
_BANDS = []
for _n, _c, _s in GROUPS:
    for _k in range(_n):
        _bin0 = _s + _k * _c
        _BANDS.append((_bin0 // 128, _bin0 % 128, _c))

# x prefetch batches (start tile, ntiles): tiles 0-1 alone so compute
# can start as soon as their columns land; modest batches so a hoisted
# later-tile stat op never waits long on a wide batch.
_XBATCH = [(0, 2), (2, 3), (5, 4), (9, 4), (13, 3)]
# Normalize the first tiles on the (faster, otherwise idle) vector
# engine: the gpsimd queue then opens with tile N's work and the first
# transposes don't sit behind a serial gpsimd chain.
_VEC_NORM_TILES = 1

_STATE = {}


def _build(has_bias):
    """Trace + compile the Bass kernel (cached per process)."""
    from contextlib import ExitStack

    import concourse.bass as bass
    import concourse.tile as tile
    from concourse import bacc, mybir

    f32 = mybir.dt.float32
    bf16 = mybir.dt.bfloat16
    nc = bacc.Bacc(
        "TRN2", target_bir_lowering=False, debug=False, num_devices=N_CORES
    )
    # x arrives already in the SBUF image layout [128, 16*257].
    x_d = nc.dram_tensor("xp", [P, N_TILES * F], f32, kind="ExternalInput").ap()
    # Weights and the normalized activations feeding the PE are bf16:
    # LN stats and PSUM accumulation stay fp32, so the only rounding is
    # on the matmul operands (~1e-3 max-rel on the output, gate is
    # 2e-2).  Halves the weight load and speeds PE transposes.
    w_d = nc.dram_tensor("wpack", [P, OUT_COLS], bf16, kind="ExternalInput").ap()
    id_d = nc.dram_tensor("ident", [P, P], bf16, kind="ExternalInput").ap()
    if has_bias:
        b_d = nc.dram_tensor("bias", [1, OUT_COLS], f32, kind="ExternalInput").ap()
    out_d = nc.dram_tensor("out", [TOK, OUT_COLS], f32, kind="ExternalOutput").ap()

    with tile.TileContext(nc) as tc, ExitStack() as ctx:
        const = ctx.enter_context(tc.tile_pool(name="const", bufs=1))
        xin = ctx.enter_context(tc.tile_pool(name="xin", bufs=1))
        sqp = ctx.enter_context(tc.tile_pool(name="sqp", bufs=3))
        xnbp = ctx.enter_context(tc.tile_pool(name="xnbp", bufs=3))
        ln = ctx.enter_context(tc.tile_pool(name="ln", bufs=3))
        xnt = ctx.enter_context(tc.tile_pool(name="xnt", bufs=3))
        outp = ctx.enter_context(tc.tile_pool(name="outp", bufs=5))
        ps_tr = ctx.enter_context(tc.tile_pool(name="ps_tr", bufs=2, space="PSUM"))
        ps_mm = ctx.enter_context(tc.tile_pool(name="ps_mm", bufs=3, space="PSUM"))

        # All of x stays resident (16.4 KB/partition): tile t of 128
        # tokens lives at xall[:, t, :].
        xall = xin.tile([P, N_TILES, F], f32)

        # The tile scheduler's cost model charges a DMA's full transfer
        # time to the ISSUING engine, so a long load queue on scalar
        # makes everything downstream of scalar (sqrt -> recip -> norm)
        # sim-late and the scheduler then buries tile 0's critical ops
        # deep in the vector queue.  Scalar gets only the two loads tile
        # 0 needs first; everything else rides sync ahead of the stores.
        def load_x(t0, nt, engine):
            engine.dma_start(
                out=xall[:, t0 : t0 + nt, :],
                in_=x_d[:, t0 * F : (t0 + nt) * F].rearrange(
                    "p (a f) -> p a f", a=nt
                ),
            )

        load_x(*_XBATCH[0], nc.scalar)
        ident = const.tile([P, P], bf16)
        nc.scalar.dma_start(out=ident[:], in_=id_d)
        w_sbr = const.tile([P, OUT_COLS], bf16)
        nc.scalar.dma_start(out=w_sbr[:, 0:1024], in_=w_d[:, 0:1024])
        load_x(*_XBATCH[1], nc.sync)
        nc.sync.dma_start(out=w_sbr[:, 1024:2048], in_=w_d[:, 1024:2048])
        load_x(*_XBATCH[2], nc.sync)
        nc.sync.dma_start(out=w_sbr[:, 2048:3072], in_=w_d[:, 2048:3072])
        load_x(*_XBATCH[3], nc.sync)
        nc.sync.dma_start(out=w_sbr[:, 3072:3584], in_=w_d[:, 3072:3584])
        load_x(*_XBATCH[4], nc.sync)
        if has_bias:
            bias_sb = const.tile([P, OUT_COLS], f32)
            nc.scalar.dma_start(
                out=bias_sb[:],
                in_=bass.AP(
                    tensor=b_d.tensor, offset=b_d.offset, ap=[[0, P], b_d.ap[1]]
                ),
            )

        eps_t = const.tile([P, 1], f32)
        nc.vector.memset(eps_t[:], EPS)
        # 1/c per band (twice: for sums and sumsq): three constants in a
        # fixed band pattern — built with memsets, no DMA on this path.
        cinv2 = const.tile([P, 2, NB], f32)
        b0 = 0
        for n, c, _s in GROUPS:
            nc.vector.memset(cinv2[:, :, b0 : b0 + n], 1.0 / c)
            b0 += n
        cinv2 = cinv2.rearrange("p a b -> p (a b)")

        # Touch the Sqrt activation table during the preamble: the ACT
        # engine's first Sqrt otherwise pays a 1.3us table load right on
        # tile 0's critical path.
        warm = const.tile([P, 1], f32)
        nc.scalar.activation(
            out=warm[:],
            in_=eps_t[:],
            func=mybir.ActivationFunctionType.Sqrt,
            bias=eps_t[:],
            scale=1.0,
        )

        for it in range(N_TILES):
            t0 = it * P
            tn = min(P, TOK - t0)

            xt = xall[:tn, it, :]
            norm_eng = nc.vector if it < _VEC_NORM_TILES else nc.gpsimd

            # --- layernorm statistics (per token x band) ---
            sq = sqp.tile([P, 256], f32)
            nc.gpsimd.tensor_mul(sq[:tn, :], xt[:, 0:256], xt[:, 0:256])

            ss = ln.tile([P, 2, NB], f32)
            b0 = 0
            for n, c, s in GROUPS:
                xg = xt[:, s : s + n * c].rearrange("p (g c) -> p g c", g=n)
                sg = sq[:tn, s : s + n * c].rearrange("p (g c) -> p g c", g=n)
                nc.vector.reduce_sum(
                    out=ss[:tn, 0, b0 : b0 + n], in_=xg, axis=mybir.AxisListType.X
                )
                nc.vector.reduce_sum(
                    out=ss[:tn, 1, b0 : b0 + n], in_=sg, axis=mybir.AxisListType.X
                )
                b0 += n

            me = ln.tile([P, 2, NB], f32)  # me[:,0]=mean, me[:,1]=E[x^2]
            nc.vector.tensor_mul(
                me[:tn].rearrange("p a b -> p (a b)"),
                ss[:tn].rearrange("p a b -> p (a b)"),
                cinv2[:tn],
            )
            mean = me[:, 0]
            var = ln.tile([P, NB], f32)
            nc.vector.tensor_mul(var[:tn, :], mean[:tn, :], mean[:tn, :])
            nc.vector.tensor_sub(var[:tn, :], me[:tn, 1, :], var[:tn, :])
            rstd = ln.tile([P, NB], f32)
            nc.scalar.activation(
                out=rstd[:tn, :],
                in_=var[:tn, :],
                func=mybir.ActivationFunctionType.Sqrt,
                bias=eps_t[:tn, :],
                scale=1.0,
            )
            nc.vector.reciprocal(out=rstd[:tn, :], in_=rstd[:tn, :])

            # --- normalize: xn = (x - mean) * rstd, cast to bf16 ---
            xnb = xnbp.tile([P, 256], bf16)
            b0 = 0
            for n, c, s in GROUPS:
                xg = xt[:, s : s + n * c].rearrange("p (g c) -> p g c", g=n)
                ng = xnb[:tn, s : s + n * c].rearrange("p (g c) -> p g c", g=n)
                norm_eng.tensor_sub(
                    xg, xg, mean[:tn, b0 : b0 + n].to_broadcast((tn, n, c))
                )
                norm_eng.tensor_mul(
                    ng, xg, rstd[:tn, b0 : b0 + n].to_broadcast((tn, n, c))
                )
                b0 += n

            # --- transpose to [bins, tok] (two 128-col halves) ---
            xnt_h = []
            for h in range(2):
                pt = ps_tr.tile([P, P], bf16, tag="pt")
                nc.tensor.transpose(
                    pt[:, :tn], xnb[:tn, h * P : (h + 1) * P], ident[:tn, :tn]
                )
                st = xnt.tile([P, P], bf16, tag=f"xnt{h}")
                nc.scalar.copy(st[:, :tn], pt[:, :tn])
                xnt_h.append(st)

            # --- 7 fp32r matmuls in 2-bank PSUM pairs + drains ---
            # one output store per 2 pairs; sync queue carries only stores.
            ot = outp.tile([P, OUT_COLS], f32)
            for pair in range(4):
                js = [j for j in (2 * pair, 2 * pair + 1) if j < N_CHUNKS]
                pm = ps_mm.tile([P, 2 * N_CHUNK], f32, tag="pm")
                for k, j in enumerate(js):
                    lhsT = xnt_h[0] if j * N_CHUNK < 2560 else xnt_h[1]
                    wcol = j * N_CHUNK
                    nc.tensor.matmul(
                        pm[:tn, k * N_CHUNK : (k + 1) * N_CHUNK],
                        lhsT[:, :tn],
                        w_sbr[:, wcol : wcol + N_CHUNK],
                        start=True,
                        stop=True,
                    )
                c0 = 2 * pair * N_CHUNK
                c1 = c0 + len(js) * N_CHUNK
                osl = ot[:tn, c0:c1]
                pms = pm[:tn, 0 : (c1 - c0)]
                if has_bias:
                    nc.vector.tensor_add(osl, pms, bias_sb[:tn, c0:c1])
                elif pair % 2 == 0:
                    nc.scalar.copy(osl, pms)
                else:
                    nc.vector.tensor_copy(osl, pms)
                # Store granularity: one DMA instruction lands on one DMA
                # engine (~26 GB/s), so aggregate bandwidth needs many
                # stores in flight.  Mid-stream that happens naturally;
                # for the last tiles the stores are split finer (and onto
                # both HWDGE queues) so the tail doesn't crawl on a
                # single engine after compute finishes.
                if 0 < it < N_TILES - 2:
                    if pair % 2 == 1:  # one store per 2 pairs
                        h0 = (pair - 1) * 2 * N_CHUNK
                        nc.sync.dma_start(
                            out=out_d[t0 : t0 + tn, h0:c1], in_=ot[:tn, h0:c1]
                        )
                elif it in (0, N_TILES - 2):  # one store per pair
                    nc.sync.dma_start(
                        out=out_d[t0 : t0 + tn, c0:c1], in_=ot[:tn, c0:c1]
                    )
                else:  # last tile: two stores per pair, alternating queues
                    for k in range(len(js)):
                        s0 = c0 + k * N_CHUNK
                        s1 = s0 + N_CHUNK
                        eng = nc.sync if (pair + k) % 2 == 0 else nc.scalar
                        eng.dma_start(
                            out=out_d[t0 : t0 + tn, s0:s1], in_=ot[:tn, s0:s1]
                        )

    nc.compile()
    return nc


def _get_nc(has_bias):
    key = ("nc", has_bias)
    if key not in _STATE:
        _STATE[key] = _build(has_bias)
    return _STATE[key]


def _pack_weights(inputs):
    """Fold gamma into W, beta/b into bias; pack block-diagonal [128, 3584]."""
    wpack = np.zeros((P, OUT_COLS), dtype=np.float32)
    bias = np.zeros((OUT_COLS,), dtype=np.float32)
    bi = 0
    for gi, (n, c, _s) in enumerate(GROUPS, start=1):
        gamma = np.asarray(inputs[f"gamma{gi}"], dtype=np.float32)  # [n, c]
        beta = np.asarray(inputs[f"beta{gi}"], dtype=np.float32)  # [n, c]
        W = np.asarray(inputs[f"W{gi}"], dtype=np.float32)  # [n, c, D]
        b = np.asarray(inputs[f"b{gi}"], dtype=np.float32)  # [n, D]
        for k in range(n):
            _ktile, krow0, cc = _BANDS[bi]
            assert cc == c
            c0, c1 = bi * D, (bi + 1) * D
            wpack[krow0 : krow0 + c, c0:c1] = gamma[k][:, None] * W[k]
            bias[c0:c1] = beta[k] @ W[k] + b[k]
            bi += 1
    return wpack, bias


def _pack_x(xflat):
    """[2000, 257] token-major -> SBUF image [128, 16*257]."""
    xp = np.zeros((P, N_TILES, F), dtype=np.float32)
    full = (TOK // P) * P  # 1920
    xp[:, : TOK // P, :] = xflat[:full].reshape(TOK // P, P, F).transpose(1, 0, 2)
    xp[: TOK - full, TOK // P, :] = xflat[full:]
    return np.ascontiguousarray(xp.reshape(P, N_TILES * F))


def _prepare(inputs):
    """-> (nc, in_maps) for the 8 cores."""
    x = np.asarray(inputs["inputs"], dtype=np.float32)
    assert x.shape == (B, T, F), x.shape
    wpack, bias = _pack_weights(inputs)
    has_bias = bool(np.any(bias != 0.0))

    nc = _get_nc(has_bias)

    import ml_dtypes

    xflat = np.ascontiguousarray(x.reshape(B * T, F))
    wpack = wpack.astype(ml_dtypes.bfloat16)
    ident = np.eye(P).astype(ml_dtypes.bfloat16)
    in_maps = []
    for c in range(N_CORES):
        m = {
            "xp": _pack_x(xflat[c * TOK : (c + 1) * TOK]),
            "wpack": wpack,
            "ident": ident,
        }
        if has_bias:
            m["bias"] = bias.reshape(1, OUT_COLS)
        in_maps.append(m)
    return nc, in_maps


def kernel(**inputs):
    from concourse.bass_utils import run_bass_kernel_spmd

    nc, in_maps = _prepare(inputs)
    res = run_bass_kernel_spmd(nc, in_maps, list(range(N_CORES))).results
    out = np.concatenate([r["out"] for r in res], axis=0)
    return out.reshape(B, T, NB, D)


# revision 26
# speedup vs baseline: 1.0872x; 1.0872x over previous
"""BandSplit layer Trainium2 kernel.

Computes, for input [16, 1000, 257]:
  - 28 frequency bands: 8 bands x 4 bins (bins 0..31), 12 x 8 (32..127),
    8 x 16 (128..255)  (bin 256 unused)
  - per-band layernorm over the band's bins (eps=1e-3), with per-band
    gamma/beta, then a per-band dense projection [c] -> [128] plus bias.
  - output [16, 1000, 28, 128]

Strategy: data-parallel over batch across 8 NeuronCores (2 batches =
2000 tokens per core).  gamma is folded into the dense weights and
beta/bias into a single per-output bias on the host, so the device does
plain layernorm + matmul.

DMA choreography (the kernel is HBM-write bound at ~400 GB/s/core):
  - x is repacked host-side into the exact SBUF image [128, 16, 257]
    so the prefetch runs in 3 DMAs with multi-KB descriptors (tiles 0-1
    first so compute starts early),
  - the scalar (Activation) HWDGE queue carries all loads, the sync
    queue carries only output stores (2 half-tile stores per 128-token
    tile) so the store stream is never stuck behind load issues,
  - per-band 1/c constants are built with memsets, not DMA.
Per 128-token tile: LN stats via free-dim reduces (DVE) -> normalize in
place (vector for the first tiles to shortcut the startup dependency
chain, GpSimd after) -> PE transpose to [bins, tok] -> 7 fp32r matmuls
(K=128 against the packed block-diagonal weights, N=512) in 2-bank
PSUM pairs -> drains alternating Scalar/Vector -> half-tile DMAs out.
"""

import sys

import numpy as np

for _p in ("/opt/trn_rl_repo", "/root/.axon_site/_ro/trn_rl_repo"):
    if _p not in sys.path:
        sys.path.append(_p)

EPS = 1e-3
D = 128
GROUPS = [(8, 4, 0), (12, 8, 32), (8, 16, 128)]  # (n_bands, bins_per_band, start_bin)
B, T, F = 16, 1000, 257
N_CORES = 8
TOK = B * T // N_CORES  # tokens per core = 2000
NB = sum(n for n, _, _ in GROUPS)  # 28 bands
OUT_COLS = NB * D  # 3584
P = 128
N_TILES = (TOK + P - 1) // P  # 16 (last tile holds 80 tokens)
N_CHUNK = 512  # matmul free-dim chunk (one PSUM bank)
N_CHUNKS = OUT_COLS // N_CHUNK  # 7
# Per-band layout: (ktile, krow0, c) per band; ktile 0 = bins 0..127,
# ktile 1 = bins 128..255. Output cols for band i are [i*128, (i+1)*128).
_BANDS = []
for _n, _c, _s in GROUPS:
    for _k in range(_n):
        _bin0 = _s + _k * _c
        _BANDS.append((_bin0 // 128, _bin0 % 128, _c))

# x prefetch batches (start tile, ntiles): tiles 0-1 alone so compute
# can start as soon as their columns land; modest batches so a hoisted
# later-tile stat op never waits long on a wide batch.
_XBATCH = [(0, 2), (2, 3), (5, 4), (9, 4), (13, 3)]
# Normalize the first tiles on the (faster, otherwise idle) vector
# engine: the gpsimd queue then opens with tile N's work and the first
# transposes don't sit behind a serial gpsimd chain.
_VEC_NORM_TILES = 1

_STATE = {}


def _build(has_bias):
    """Trace + compile the Bass kernel (cached per process)."""
    from contextlib import ExitStack

    import concourse.bass as bass
    import concourse.tile as tile
    from concourse import bacc, mybir

    f32 = mybir.dt.float32
    f32r = mybir.dt.float32r
    nc = bacc.Bacc(
        "TRN2", target_bir_lowering=False, debug=False, num_devices=N_CORES
    )
    # x arrives already in the SBUF image layout [128, 16*257].
    x_d = nc.dram_tensor("xp", [P, N_TILES * F], f32, kind="ExternalInput").ap()
    # Declared float32r (same 4-byte layout): DMA straight to the fp32r
    # weight tile with no on-chip rounding pass.
    w_d = nc.dram_tensor("wpack", [P, OUT_COLS], f32r, kind="ExternalInput").ap()
    id_d = nc.dram_tensor("ident", [P, P], f32, kind="ExternalInput").ap()
    if has_bias:
        b_d = nc.dram_tensor("bias", [1, OUT_COLS], f32, kind="ExternalInput").ap()
    out_d = nc.dram_tensor("out", [TOK, OUT_COLS], f32, kind="ExternalOutput").ap()

    with tile.TileContext(nc) as tc, ExitStack() as ctx:
        const = ctx.enter_context(tc.tile_pool(name="const", bufs=1))
        xin = ctx.enter_context(tc.tile_pool(name="xin", bufs=1))
        sqp = ctx.enter_context(tc.tile_pool(name="sqp", bufs=3))
        ln = ctx.enter_context(tc.tile_pool(name="ln", bufs=3))
        xnt = ctx.enter_context(tc.tile_pool(name="xnt", bufs=3))
        outp = ctx.enter_context(tc.tile_pool(name="outp", bufs=6))
        ps_tr = ctx.enter_context(tc.tile_pool(name="ps_tr", bufs=2, space="PSUM"))
        ps_mm = ctx.enter_context(tc.tile_pool(name="ps_mm", bufs=3, space="PSUM"))

        # All of x stays resident (16.4 KB/partition): tile t of 128
        # tokens lives at xall[:, t, :].
        xall = xin.tile([P, N_TILES, F], f32)

        # The tile scheduler's cost model charges a DMA's full transfer
        # time to the ISSUING engine, so a long load queue on scalar
        # makes everything downstream of scalar (sqrt -> recip -> norm)
        # sim-late and the scheduler then buries tile 0's critical ops
        # deep in the vector queue.  Scalar gets only the two loads tile
        # 0 needs first; everything else rides sync ahead of the stores.
        def load_x(t0, nt, engine):
            engine.dma_start(
                out=xall[:, t0 : t0 + nt, :],
                in_=x_d[:, t0 * F : (t0 + nt) * F].rearrange(
                    "p (a f) -> p a f", a=nt
                ),
            )

        load_x(*_XBATCH[0], nc.scalar)
        ident = const.tile([P, P], f32)
        nc.scalar.dma_start(out=ident[:], in_=id_d)
        w_sbr = const.tile([P, OUT_COLS], f32r)
        nc.scalar.dma_start(out=w_sbr[:, 0:1024], in_=w_d[:, 0:1024])
        load_x(*_XBATCH[1], nc.sync)
        nc.sync.dma_start(out=w_sbr[:, 1024:2048], in_=w_d[:, 1024:2048])
        load_x(*_XBATCH[2], nc.sync)
        nc.sync.dma_start(out=w_sbr[:, 2048:3072], in_=w_d[:, 2048:3072])
        load_x(*_XBATCH[3], nc.sync)
        nc.sync.dma_start(out=w_sbr[:, 3072:3584], in_=w_d[:, 3072:3584])
        load_x(*_XBATCH[4], nc.sync)
        if has_bias:
            bias_sb = const.tile([P, OUT_COLS], f32)
            nc.scalar.dma_start(
                out=bias_sb[:],
                in_=bass.AP(
                    tensor=b_d.tensor, offset=b_d.offset, ap=[[0, P], b_d.ap[1]]
                ),
            )

        eps_t = const.tile([P, 1], f32)
        nc.vector.memset(eps_t[:], EPS)
        # 1/c per band (twice: for sums and sumsq): three constants in a
        # fixed band pattern — built with memsets, no DMA on this path.
        cinv2 = const.tile([P, 2, NB], f32)
        b0 = 0
        for n, c, _s in GROUPS:
            nc.vector.memset(cinv2[:, :, b0 : b0 + n], 1.0 / c)
            b0 += n
        cinv2 = cinv2.rearrange("p a b -> p (a b)")

        # Touch the Sqrt activation table during the preamble: the ACT
        # engine's first Sqrt otherwise pays a 1.3us table load right on
        # tile 0's critical path.
        warm = const.tile([P, 1], f32)
        nc.scalar.activation(
            out=warm[:],
            in_=eps_t[:],
            func=mybir.ActivationFunctionType.Sqrt,
            bias=eps_t[:],
            scale=1.0,
        )

        for it in range(N_TILES):
            t0 = it * P
            tn = min(P, TOK - t0)

            xt = xall[:tn, it, :]
            norm_eng = nc.vector if it < _VEC_NORM_TILES else nc.gpsimd

            # --- layernorm statistics (per token x band) ---
            sq = sqp.tile([P, 256], f32)
            nc.gpsimd.tensor_mul(sq[:tn, :], xt[:, 0:256], xt[:, 0:256])

            ss = ln.tile([P, 2, NB], f32)
            b0 = 0
            for n, c, s in GROUPS:
                xg = xt[:, s : s + n * c].rearrange("p (g c) -> p g c", g=n)
                sg = sq[:tn, s : s + n * c].rearrange("p (g c) -> p g c", g=n)
                nc.vector.reduce_sum(
                    out=ss[:tn, 0, b0 : b0 + n], in_=xg, axis=mybir.AxisListType.X
                )
                nc.vector.reduce_sum(
                    out=ss[:tn, 1, b0 : b0 + n], in_=sg, axis=mybir.AxisListType.X
                )
                b0 += n

            me = ln.tile([P, 2, NB], f32)  # me[:,0]=mean, me[:,1]=E[x^2]
            nc.vector.tensor_mul(
                me[:tn].rearrange("p a b -> p (a b)"),
                ss[:tn].rearrange("p a b -> p (a b)"),
                cinv2[:tn],
            )
            mean = me[:, 0]
            var = ln.tile([P, NB], f32)
            nc.vector.tensor_mul(var[:tn, :], mean[:tn, :], mean[:tn, :])
            nc.vector.tensor_sub(var[:tn, :], me[:tn, 1, :], var[:tn, :])
            rstd = ln.tile([P, NB], f32)
            nc.scalar.activation(
                out=rstd[:tn, :],
                in_=var[:tn, :],
                func=mybir.ActivationFunctionType.Sqrt,
                bias=eps_t[:tn, :],
                scale=1.0,
            )
            nc.vector.reciprocal(out=rstd[:tn, :], in_=rstd[:tn, :])

            # --- normalize in place: xn = (x - mean) * rstd ---
            b0 = 0
            for n, c, s in GROUPS:
                xg = xt[:, s : s + n * c].rearrange("p (g c) -> p g c", g=n)
                norm_eng.tensor_sub(
                    xg, xg, mean[:tn, b0 : b0 + n].to_broadcast((tn, n, c))
                )
                norm_eng.tensor_mul(
                    xg, xg, rstd[:tn, b0 : b0 + n].to_broadcast((tn, n, c))
                )
                b0 += n

            # --- transpose to [bins, tok] (two 128-col halves) ---
            xnt_h = []
            for h in range(2):
                pt = ps_tr.tile([P, P], f32, tag="pt")
                nc.tensor.transpose(
                    pt[:, :tn], xt[:, h * P : (h + 1) * P], ident[:tn, :tn]
                )
                st = xnt.tile([P, P], f32r, tag=f"xnt{h}")
                nc.scalar.copy(st[:, :tn], pt[:, :tn])
                xnt_h.append(st)

            # --- 7 fp32r matmuls in 2-bank PSUM pairs + drains ---
            # one output store per 2 pairs; sync queue carries only stores.
            ot = outp.tile([P, OUT_COLS], f32)
            for pair in range(4):
                js = [j for j in (2 * pair, 2 * pair + 1) if j < N_CHUNKS]
                pm = ps_mm.tile([P, 2 * N_CHUNK], f32, tag="pm")
                for k, j in enumerate(js):
                    lhsT = xnt_h[0] if j * N_CHUNK < 2560 else xnt_h[1]
                    wcol = j * N_CHUNK
                    nc.tensor.matmul(
                        pm[:tn, k * N_CHUNK : (k + 1) * N_CHUNK],
                        lhsT[:, :tn],
                        w_sbr[:, wcol : wcol + N_CHUNK],
                        start=True,
                        stop=True,
                    )
                c0 = 2 * pair * N_CHUNK
                c1 = c0 + len(js) * N_CHUNK
                osl = ot[:tn, c0:c1]
                pms = pm[:tn, 0 : (c1 - c0)]
                if has_bias:
                    nc.vector.tensor_add(osl, pms, bias_sb[:tn, c0:c1])
                elif pair % 2 == 0:
                    nc.scalar.copy(osl, pms)
                else:
                    nc.vector.tensor_copy(osl, pms)
                # Store granularity: one DMA instruction lands on one DMA
                # engine (~26 GB/s), so aggregate bandwidth needs many
                # stores in flight.  Mid-stream that happens naturally;
                # for the last tiles the stores are split finer (and onto
                # both HWDGE queues) so the tail doesn't crawl on a
                # single engine after compute finishes.
                if it in (0, 1, N_TILES - 3, N_TILES - 2):
                    # one store per pair where the pipeline is shallow
                    nc.sync.dma_start(
                        out=out_d[t0 : t0 + tn, c0:c1], in_=ot[:tn, c0:c1]
                    )
                elif it < N_TILES - 1:
                    if pair % 2 == 1:  # one store per 2 pairs
                        h0 = (pair - 1) * 2 * N_CHUNK
                        nc.sync.dma_start(
                            out=out_d[t0 : t0 + tn, h0:c1], in_=ot[:tn, h0:c1]
                        )
                else:  # last tile: two stores per pair, alternating queues
                    for k in range(len(js)):
                        s0 = c0 + k * N_CHUNK
                        s1 = s0 + N_CHUNK
                        eng = nc.sync if (pair + k) % 2 == 0 else nc.scalar
                        eng.dma_start(
                            out=out_d[t0 : t0 + tn, s0:s1], in_=ot[:tn, s0:s1]
                        )

    nc.compile()
    return nc


def _get_nc(has_bias):
    key = ("nc", has_bias)
    if key not in _STATE:
        _STATE[key] = _build(has_bias)
    return _STATE[key]


def _pack_weights(inputs):
    """Fold gamma into W, beta/b into bias; pack block-diagonal [128, 3584]."""
    wpack = np.zeros((P, OUT_COLS), dtype=np.float32)
    bias = np.zeros((OUT_COLS,), dtype=np.float32)
    bi = 0
    for gi, (n, c, _s) in enumerate(GROUPS, start=1):
        gamma = np.asarray(inputs[f"gamma{gi}"], dtype=np.float32)  # [n, c]
        beta = np.asarray(inputs[f"beta{gi}"], dtype=np.float32)  # [n, c]
        W = np.asarray(inputs[f"W{gi}"], dtype=np.float32)  # [n, c, D]
        b = np.asarray(inputs[f"b{gi}"], dtype=np.float32)  # [n, D]
        for k in range(n):
            _ktile, krow0, cc = _BANDS[bi]
            assert cc == c
            c0, c1 = bi * D, (bi + 1) * D
            wpack[krow0 : krow0 + c, c0:c1] = gamma[k][:, None] * W[k]
            bias[c0:c1] = beta[k] @ W[k] + b[k]
            bi += 1
    return wpack, bias


def _pack_x(xflat):
    """[2000, 257] token-major -> SBUF image [128, 16*257]."""
    xp = np.zeros((P, N_TILES, F), dtype=np.float32)
    full = (TOK // P) * P  # 1920
    xp[:, : TOK // P, :] = xflat[:full].reshape(TOK // P, P, F).transpose(1, 0, 2)
    xp[: TOK - full, TOK // P, :] = xflat[full:]
    return np.ascontiguousarray(xp.reshape(P, N_TILES * F))


def _prepare(inputs):
    """-> (nc, in_maps) for the 8 cores."""
    x = np.asarray(inputs["inputs"], dtype=np.float32)
    assert x.shape == (B, T, F), x.shape
    wpack, bias = _pack_weights(inputs)
    has_bias = bool(np.any(bias != 0.0))

    nc = _get_nc(has_bias)

    xflat = np.ascontiguousarray(x.reshape(B * T, F))
    ident = np.eye(P, dtype=np.float32)
    in_maps = []
    for c in range(N_CORES):
        m = {
            "xp": _pack_x(xflat[c * TOK : (c + 1) * TOK]),
            "wpack": wpack,
            "ident": ident,
        }
        if has_bias:
            m["bias"] = bias.reshape(1, OUT_COLS)
        in_maps.append(m)
    return nc, in_maps


def kernel(**inputs):
    from concourse.bass_utils import run_bass_kernel_spmd

    nc, in_maps = _prepare(inputs)
    res = run_bass_kernel_spmd(nc, in_maps, list(range(N_CORES))).results
    out = np.concatenate([r["out"] for r in res], axis=0)
    return out.reshape(B, T, NB, D)
